# revision 15
# baseline (speedup 1.0000x reference)
"""Bass/Trainium2 kernel for nn_BitwiseTasNetRepeat.

Strategy (v4: STT-fused center tap + STT-fused residual correction)
-------------------------------------------------------------------
Each sign(BN(.)) collapses to a per-channel threshold compare. Per block:

    S1 = (R >= t1)                          {0,1} fp8  (DVE is_ge, 2x mode)
    p1 = W1s @ S1                           (TensorE fp8 DR, K=256)
    N2 = Sign(-2*p1 + (rs1+t2)) = -sign(bn2)  (ACT, fp8 +-1)
    U-path (S3 on DVE):
      q0 = -a0*N2(-d) - a2*N2(+d)           (ONE diag DR matmul, taps +-d)
      S3 = (q0 - tau3 >= N2(0))             {0,1}  (DVE scalar_tensor_tensor)
    V-path (S3 on ACT):
      qv = -a0*N2(-d) - N2(0) - a2*N2(+d)   (diag DR + diag plain matmul)
      S3 = Sign(qv - tau3)                  +-1    (ACT)
    ps2 = W2x @ S3                          (TensorE fp8 DR, K=512)
    R   = (ps2 + rneg) + R                  (DVE scalar_tensor_tensor;
                                             rneg corrects U's {0,1} encoding)

All values exact in fp8e4m3 / fp32-PSUM; result is bit-exact.
Sharding: data-parallel over batch, 2 batches per core, 8 cores.
Loops are ordered (stage-major, b-outer, mh-outer) so TensorE reuses
LDWEIGHTS across 8-16 matmuls and every engine FIFO stays dependency-
feasible in emission order.
"""

import numpy as np
import ml_dtypes

_B, _CB, _H, _T = 16, 256, 512, 4096
_BLOCKS = 8
_EPS = 1e-5
_NCORES = 8
_BS = _B // _NCORES      # batches per core
_KC = _CB // 128         # 2  k-tiles of Cb
_MH = _H // 128          # 4  m-tiles of H
_PAD = 128               # halo for dilated depthwise conv (max d = 128)
_NCC = 12                # f32 const columns per block
_QC = 1024               # chunk width

# --- engine assignment knobs (tunable) ---------------------------------
def _isV(i, mh):
    """True -> S3 of tile (i, mh) runs on ACT via the 3-tap matmul path
    ("V"); False -> 2-tap matmul + DVE scalar_tensor_tensor ("U")."""
    return (mh < 2) if i % 2 == 0 else (mh == 2)


def _isN2dve(i, mh):
    """True -> N2 of tile (i, mh) produced on DVE as {0,1} (is_lt);
    False -> on ACT as +-1 (Sign)."""
    return False

_nc_cache = {}


def _mk3(ap2d, j_step, cols):
    """3D AP [128, 2 (stride j_step), cols] over a 2D row view."""
    import bass_rust
    v = ap2d.copy()
    l = v.ap
    v.ap = bass_rust.VecI64Pair([list(l[0]), [j_step, 2], [1, cols]])
    return v


def _build_nc(bs=_BS, nblocks=_BLOCKS, T=_T):
    import concourse.mybir as mybir
    from concourse import bacc
    from concourse.tile import TileContext

    f32 = mybir.dt.float32
    fp8 = mybir.dt.float8e4
    ALU = mybir.AluOpType
    ACTF = mybir.ActivationFunctionType
    DRM = mybir.MatmulPerfMode.DoubleRow
    nq = T // _QC

    nc = bacc.Bacc("TRN2", target_bir_lowering=False, debug=False,
                   enable_asserts=False)

    x_d = nc.dram_tensor("x", [bs, _CB, T], f32, kind="ExternalInput")
    w1_d = nc.dram_tensor("w1dr", [128, nblocks * _MH * 256], fp8,
                          kind="ExternalInput")
    w2_d = nc.dram_tensor("w2dr", [128, nblocks * _KC * 2 * 256], fp8,
                          kind="ExternalInput")
    dwa_d = nc.dram_tensor("dwA", [128, nblocks * _MH * 256], fp8,
                           kind="ExternalInput")
    dwb_d = nc.dram_tensor("dwB", [128, nblocks * _MH * 128], fp8,
                           kind="ExternalInput")
    cst_d = nc.dram_tensor("cst", [128, nblocks * _NCC], f32,
                           kind="ExternalInput")
    out_d = nc.dram_tensor("out", [bs, _CB, T], f32, kind="ExternalOutput")

    with TileContext(nc) as tc:
        with (
            tc.tile_pool(name="wpool", bufs=1) as wpool,
            tc.tile_pool(name="rpool", bufs=4) as rpool,
            tc.tile_pool(name="s1pool", bufs=3) as s1pool,
            tc.tile_pool(name="n2pool", bufs=12) as n2pool,
            tc.tile_pool(name="s3pool", bufs=8) as s3pool,
            tc.tile_pool(name="psmm", bufs=2, space="PSUM") as psmm,
            tc.tile_pool(name="psdw", bufs=2, space="PSUM") as psdw,
        ):
            w1sb = wpool.tile([128, nblocks * _MH * 256], fp8)
            nc.sync.dma_start(out=w1sb[:], in_=w1_d.ap())
            w2sb = wpool.tile([128, nblocks * _KC * 2 * 256], fp8)
            nc.sync.dma_start(out=w2sb[:], in_=w2_d.ap())
            dwasb = wpool.tile([128, nblocks * _MH * 256], fp8)
            nc.sync.dma_start(out=dwasb[:], in_=dwa_d.ap())
            dwbsb = wpool.tile([128, nblocks * _MH * 128], fp8)
            nc.sync.dma_start(out=dwbsb[:], in_=dwb_d.ap())
            cst = wpool.tile([128, nblocks * _NCC], f32)
            nc.sync.dma_start(out=cst[:], in_=cst_d.ap())

            def w1t(i, mh):
                o = (i * _MH + mh) * 256
                return _mk3(w1sb[:, o:o + 256], 128, 128)

            def w2t(i, mc, pair):
                o = (i * _KC * 2 + mc * 2 + pair) * 256
                return _mk3(w2sb[:, o:o + 256], 128, 128)

            def dwAt(i, mh):
                o = (i * _MH + mh) * 256
                return _mk3(dwasb[:, o:o + 256], 128, 128)

            def dwBt(i, mh):
                o = (i * _MH + mh) * 128
                return dwbsb[:, o:o + 128]

            def cc(i, j):
                return cst[:, i * _NCC + j:i * _NCC + j + 1]

            Rb = {}
            for b in range(bs):
                Rb[b] = []
                for kc in range(_KC):
                    rt = rpool.tile([128, T], f32, tag="R",
                                    name=f"R_b{b}_{kc}")
                    nc.sync.dma_start(
                        out=rt[:], in_=x_d.ap()[b, kc * 128:(kc + 1) * 128, :])
                    Rb[b].append(rt)

            state = {}

            def emitA_alloc(b, i):
                s1 = s1pool.tile([128, _KC * T], fp8, tag="S1",
                                 name=f"S1_b{b}_i{i}")
                N2 = []
                for mh in range(_MH):
                    n2 = n2pool.tile([128, T + 2 * _PAD], fp8, tag="N2",
                                     name=f"N2_b{b}_i{i}_{mh}")
                    hv = 0.5 if _isN2dve(i, mh) else 0.0
                    nc.gpsimd.memset(n2[:, 0:_PAD], hv)
                    nc.gpsimd.memset(n2[:, _PAD + T:2 * _PAD + T], hv)
                    N2.append(n2)
                state[(b, i)] = (s1, N2)

            def emitA_g(b, i, g):
                R = Rb[b]
                s1, N2 = state[(b, i)]
                c0 = g * _QC
                for kc in range(_KC):
                    nc.gpsimd.tensor_scalar(
                        s1[:, kc * T + c0:kc * T + c0 + _QC],
                        R[kc][:, c0:c0 + _QC],
                        cc(i, kc), None, op0=ALU.is_ge)
                for mh in range(_MH):
                    ps = psmm.tile([128, _QC], f32, tag="mm",
                                   name=f"psA_{b}_{i}_{mh}_{g}")
                    for nn in range(2):
                        cn = c0 + nn * 512
                        rhs = _mk3(s1[:, cn:cn + 512], T, 512)
                        nc.tensor.matmul(
                            ps[:, nn * 512:(nn + 1) * 512],
                            w1t(i, mh), rhs, start=True, stop=True,
                            perf_mode=DRM)
                    dst = N2[mh][:, _PAD + c0:_PAD + c0 + _QC]
                    if _isN2dve(i, mh):
                        nc.vector.tensor_scalar(
                            dst, ps[:], cc(i, 2 + mh), None, op0=ALU.is_lt)
                    else:
                        nc.scalar.activation(
                            dst, ps[:], ACTF.Sign,
                            bias=cc(i, 2 + mh), scale=-2.0)

            def emitBC_q(b, i, q):
                d = 2 ** i
                R = Rb[b]
                _, N2 = state[(b, i)]
                c0 = q * _QC
                S3 = [s3pool.tile([128, 2 * _QC], fp8, tag="S3",
                                  name=f"S3_b{b}_i{i}_q{q}_p{p}")
                      for p in range(2)]
                for mh in range(_MH):
                    n2 = N2[mh]
                    pd = psdw.tile([128, _QC], f32, tag="dw",
                                   name=f"psD_{b}_{i}_{mh}_{q}")
                    if _isV(i, mh):
                        # grouped A,A,B,B so LDWEIGHTS is reused per pair
                        for nn in range(2):
                            w0 = _PAD + c0 + nn * 512
                            rhs01 = _mk3(n2[:, w0 - d:w0 - d + 512], d, 512)
                            nc.tensor.matmul(
                                pd[:, nn * 512:(nn + 1) * 512],
                                dwAt(i, mh), rhs01,
                                start=True, stop=False, perf_mode=DRM)
                        for nn in range(2):
                            w0 = _PAD + c0 + nn * 512
                            nc.tensor.matmul(
                                pd[:, nn * 512:(nn + 1) * 512],
                                dwBt(i, mh), n2[:, w0 + d:w0 + d + 512],
                                start=False, stop=True)
                    else:
                        for nn in range(2):
                            w0 = _PAD + c0 + nn * 512
                            rhs02 = _mk3(n2[:, w0 - d:w0 - d + 512],
                                         2 * d, 512)
                            nc.tensor.matmul(
                                pd[:, nn * 512:(nn + 1) * 512],
                                dwAt(i, mh), rhs02,
                                start=True, stop=True, perf_mode=DRM)
                    s3out = S3[mh // 2][:, (mh % 2) * _QC:(mh % 2 + 1) * _QC]
                    if _isV(i, mh):
                        sc = 2.0 if _isN2dve(i, mh) else 1.0
                        nc.scalar.activation(
                            s3out, pd[:], ACTF.Sign,
                            bias=cc(i, 6 + mh), scale=sc)
                    else:
                        nc.vector.scalar_tensor_tensor(
                            s3out, pd[:], cc(i, 6 + mh),
                            n2[:, _PAD + c0:_PAD + c0 + _QC],
                            op0=ALU.add, op1=ALU.is_ge)
                for mc in range(_KC):
                    ps2 = psmm.tile([128, _QC], f32, tag="mm",
                                    name=f"psC_{b}_{i}_{q}_{mc}")
                    # pair-outer: reuse each LDWEIGHTS across both nn halves
                    for pair in range(2):
                        for nn in range(2):
                            rhs = _mk3(S3[pair][:, nn * 512:nn * 512 + 512],
                                       _QC, 512)
                            nc.tensor.matmul(
                                ps2[:, nn * 512:(nn + 1) * 512],
                                w2t(i, mc, pair), rhs,
                                start=(pair == 0), stop=(pair == 1),
                                perf_mode=DRM)
                    nc.vector.scalar_tensor_tensor(
                        R[mc][:, c0:c0 + _QC], ps2[:],
                        cc(i, 10 + mc), R[mc][:, c0:c0 + _QC],
                        op0=ALU.add, op1=ALU.add)

            # software-pipelined emission: stage A of unit k+1 interleaves
            # chunk-wise with stage B/C of unit k (independent units).
            # Prologue pipelines within unit 0: BC_q(0, q) only needs N2
            # chunks q-1..q+1, so it starts after A_g(0, q+1).
            seq = [(b, i) for i in range(nblocks) for b in range(bs)]
            emitA_alloc(*seq[0])
            emitA_g(*seq[0], 0)
            emitA_g(*seq[0], 1)
            emitA_alloc(*seq[1])
            emitA_g(*seq[1], 0)
            emitBC_q(*seq[0], 0)
            emitA_g(*seq[0], 2)
            emitA_g(*seq[1], 1)
            emitBC_q(*seq[0], 1)
            emitA_g(*seq[0], 3)
            emitA_g(*seq[1], 2)
            emitBC_q(*seq[0], 2)
            emitA_g(*seq[1], 3)
            emitBC_q(*seq[0], 3)
            state.pop(seq[0])
            for k in range(1, len(seq)):
                if k + 1 < len(seq):
                    emitA_alloc(*seq[k + 1])
                for g in range(nq):
                    emitBC_q(*seq[k], g)
                    if k + 1 < len(seq):
                        emitA_g(*seq[k + 1], g)
                state.pop(seq[k])

            for b in range(bs):
                for kc in range(_KC):
                    nc.sync.dma_start(
                        out=out_d.ap()[b, kc * 128:(kc + 1) * 128, :],
                        in_=Rb[b][kc][:])
    nc.finalize()
    return nc


def _prep(inputs, nblocks=_BLOCKS):
    """Host-side weight/threshold preprocessing (tiny tensors only)."""
    e4 = ml_dtypes.float8_e4m3

    def thr(g, bb, m, v):
        return (m - bb * np.sqrt(v + _EPS) / g).astype(np.float32)

    w1dr = np.zeros((128, nblocks * _MH * 256), np.float32)
    w2dr = np.zeros((128, nblocks * _KC * 2 * 256), np.float32)
    dwA = np.zeros((128, nblocks * _MH * 256), np.float32)
    dwB = np.zeros((128, nblocks * _MH * 128), np.float32)
    cst = np.zeros((128, nblocks * _NCC), np.float32)
    ar = np.arange(128)
    for i in range(nblocks):
        t1 = thr(inputs['bn1_gamma'][i], inputs['bn1_beta'][i],
                 inputs['bn1_mean'][i], inputs['bn1_var'][i])          # [Cb]
        t2 = thr(inputs['bn2_gamma'][i], inputs['bn2_beta'][i],
                 inputs['bn2_mean'][i], inputs['bn2_var'][i])          # [H]
        t3 = thr(inputs['bn3_gamma'][i], inputs['bn3_beta'][i],
                 inputs['bn3_mean'][i], inputs['bn3_var'][i])          # [H]
        W1s = np.sign(inputs['w1'][i]).astype(np.float32)              # [H, Cb]
        W2s = np.sign(inputs['w2'][i]).astype(np.float32)              # [Cb, H]
        dws = np.sign(inputs['dw_w'][i]).astype(np.float32)            # [H, 3]
        ctr = dws[:, 1]
        a0 = dws[:, 0] * ctr
        a2 = dws[:, 2] * ctr
        rs1 = W1s.sum(axis=1)                                          # [H]
        tau3 = ctr * t3
        # U tiles ({0,1}-encoded S3) get amp 2; V tiles (+-1 S3) amp 1
        mh_of = np.arange(_H) // 128
        isV = np.array([_isV(i, mh) for mh in mh_of])
        amp = np.where(isV, 1.0, 2.0)
        W2x = W2s * (ctr * amp)[None, :]                               # [Cb, H]
        for mh in range(_MH):
            o = (i * _MH + mh) * 256
            for j in range(2):
                w1dr[:, o + j * 128:o + (j + 1) * 128] = \
                    W1s[mh * 128:(mh + 1) * 128, j * 128:(j + 1) * 128].T
        for mc in range(_KC):
            for pair in range(2):
                o = (i * _KC * 2 + mc * 2 + pair) * 256
                for j in range(2):
                    kh = pair * 2 + j
                    w2dr[:, o + j * 128:o + (j + 1) * 128] = \
                        W2x[mc * 128:(mc + 1) * 128,
                            kh * 128:(kh + 1) * 128].T
        for mh in range(_MH):
            sl = slice(mh * 128, (mh + 1) * 128)
            o = (i * _MH + mh) * 256
            o2 = (i * _MH + mh) * 128
            if _isV(i, mh):
                # 3-tap path: DR rows (t-d, t) coeffs (-a0, -1); plain (+d): -a2
                dwA[ar, o + ar] = -a0[sl]
                dwA[ar, o + 128 + ar] = -1.0
                dwB[ar, o2 + ar] = -a2[sl]
            else:
                # 2-tap path: DR rows (t-d, t+d) coeffs (-a0, -a2)
                dwA[ar, o + ar] = -a0[sl]
                dwA[ar, o + 128 + ar] = -a2[sl]
        base = i * _NCC
        for kc in range(_KC):
            cst[:, base + kc] = t1[kc * 128:(kc + 1) * 128]
        for mh in range(_MH):
            sl = slice(mh * 128, (mh + 1) * 128)
            nb = rs1[sl] + t2[sl]
            # N2 on ACT: Sign(-2*p1' + nb); on DVE: is_lt(p1', nb/2)
            cst[:, base + 2 + mh] = (nb / 2 if _isN2dve(i, mh) else nb)
            if _isV(i, mh):
                if _isN2dve(i, mh):
                    cst[:, base + 6 + mh] = (a0[sl] + 1 + a2[sl]) - tau3[sl]
                else:
                    cst[:, base + 6 + mh] = -tau3[sl]
            else:
                if _isN2dve(i, mh):
                    cst[:, base + 6 + mh] = \
                        (a0[sl] + a2[sl] + 1 - tau3[sl]) / 2
                else:
                    cst[:, base + 6 + mh] = -tau3[sl]
        # R correction: U channels contribute +W2s*ctr per column extra
        rn = -(W2s * (ctr * (~isV))[None, :]).sum(axis=1)              # [Cb]
        for mc in range(_KC):
            cst[:, base + 10 + mc] = rn[mc * 128:(mc + 1) * 128]
    return (w1dr.astype(e4), w2dr.astype(e4), dwA.astype(e4),
            dwB.astype(e4), cst)


def kernel(**inputs):
    inputs = {k: np.asarray(v) for k, v in inputs.items()}
    x = inputs['x'].astype(np.float32)
    w1dr, w2dr, dwA, dwB, cst = _prep(inputs)

    if 'nc' not in _nc_cache:
        _nc_cache['nc'] = _build_nc()
    nc = _nc_cache['nc']

    in_maps = []
    for c in range(_NCORES):
        in_maps.append({
            'x': np.ascontiguousarray(x[c * _BS:(c + 1) * _BS]),
            'w1dr': w1dr, 'w2dr': w2dr, 'dwA': dwA, 'dwB': dwB,
            'cst': cst,
        })

    from concourse.bass_utils import run_bass_kernel_spmd
    import os
    trace = bool(int(os.environ.get('KERNEL_TRACE', '0')))
    res = run_bass_kernel_spmd(nc, in_maps, core_ids=list(range(_NCORES)),
                               trace=trace)
    _nc_cache['last_result'] = res
    out = np.concatenate([r['out'] for r in res.results], axis=0)
    return out.astype(np.float32)


# revision 20
# speedup vs baseline: 3.5670x; 3.5670x over previous
"""Bass/Trainium2 kernel for nn_BitwiseTasNetRepeat.

Strategy (v4: STT-fused center tap + STT-fused residual correction)
-------------------------------------------------------------------
Each sign(BN(.)) collapses to a per-channel threshold compare. Per block:

    S1 = (R >= t1)                          {0,1} fp8  (DVE is_ge, 2x mode)
    p1 = W1s @ S1                           (TensorE fp8 DR, K=256)
    N2 = Sign(-2*p1 + (rs1+t2)) = -sign(bn2)  (ACT, fp8 +-1)
    U-path (S3 on DVE):
      q0 = -a0*N2(-d) - a2*N2(+d)           (ONE diag DR matmul, taps +-d)
      S3 = (q0 - tau3 >= N2(0))             {0,1}  (DVE scalar_tensor_tensor)
    V-path (S3 on ACT):
      qv = -a0*N2(-d) - N2(0) - a2*N2(+d)   (diag DR + diag plain matmul)
      S3 = Sign(qv - tau3)                  +-1    (ACT)
    ps2 = W2x @ S3                          (TensorE fp8 DR, K=512)
    R   = (ps2 + rneg) + R                  (DVE scalar_tensor_tensor;
                                             rneg corrects U's {0,1} encoding)

All values exact in fp8e4m3 / fp32-PSUM; result is bit-exact.
Sharding: data-parallel over batch, 2 batches per core, 8 cores.
Loops are ordered (stage-major, b-outer, mh-outer) so TensorE reuses
LDWEIGHTS across 8-16 matmuls and every engine FIFO stays dependency-
feasible in emission order.
"""

import numpy as np
import ml_dtypes

_B, _CB, _H, _T = 16, 256, 512, 4096
_BLOCKS = 8
_EPS = 1e-5
_NCORES = 8
_BS = _B // _NCORES      # batches per core
_KC = _CB // 128         # 2  k-tiles of Cb
_MH = _H // 128          # 4  m-tiles of H
_PAD = 128               # halo for dilated depthwise conv (max d = 128)
_NCC = 12                # f32 const columns per block
_QC = 1024               # chunk width

# --- engine assignment knobs (tunable) ---------------------------------
def _isV(i, mh):
    """True -> S3 of tile (i, mh) runs on ACT via the 3-tap matmul path
    ("V"); False -> 2-tap matmul + DVE scalar_tensor_tensor ("U")."""
    if i % 2 == 0:
        return mh < 2
    if i in (1, 3):
        return mh >= 2
    return mh == 2


def _isN2dve(i, mh):
    """True -> N2 of tile (i, mh) produced on DVE as {0,1} (is_lt);
    False -> on ACT as +-1 (Sign)."""
    return False

_nc_cache = {}


def _mk3(ap2d, j_step, cols):
    """3D AP [128, 2 (stride j_step), cols] over a 2D row view."""
    import bass_rust
    v = ap2d.copy()
    l = v.ap
    v.ap = bass_rust.VecI64Pair([list(l[0]), [j_step, 2], [1, cols]])
    return v


def _build_nc(bs=_BS, nblocks=_BLOCKS, T=_T):
    import concourse.mybir as mybir
    from concourse import bacc
    from concourse.tile import TileContext

    f32 = mybir.dt.float32
    fp8 = mybir.dt.float8e4
    ALU = mybir.AluOpType
    ACTF = mybir.ActivationFunctionType
    DRM = mybir.MatmulPerfMode.DoubleRow
    nq = T // _QC

    nc = bacc.Bacc("TRN2", target_bir_lowering=False, debug=False,
                   enable_asserts=False)

    x_d = nc.dram_tensor("x", [bs, _CB, T], f32, kind="ExternalInput")
    w1_d = nc.dram_tensor("w1dr", [128, nblocks * _MH * 256], fp8,
                          kind="ExternalInput")
    w2_d = nc.dram_tensor("w2dr", [128, nblocks * _KC * 2 * 256], fp8,
                          kind="ExternalInput")
    dwa_d = nc.dram_tensor("dwA", [128, nblocks * _MH * 256], fp8,
                           kind="ExternalInput")
    dwb_d = nc.dram_tensor("dwB", [128, nblocks * _MH * 128], fp8,
                           kind="ExternalInput")
    cst_d = nc.dram_tensor("cst", [128, nblocks * _NCC], f32,
                           kind="ExternalInput")
    out_d = nc.dram_tensor("out", [bs, _CB, T], f32, kind="ExternalOutput")

    with TileContext(nc) as tc:
        with (
            tc.tile_pool(name="wpool", bufs=1) as wpool,
            tc.tile_pool(name="rpool", bufs=4) as rpool,
            tc.tile_pool(name="s1pool", bufs=3) as s1pool,
            tc.tile_pool(name="n2pool", bufs=12) as n2pool,
            tc.tile_pool(name="s3pool", bufs=8) as s3pool,
            tc.tile_pool(name="psmm", bufs=2, space="PSUM") as psmm,
            tc.tile_pool(name="psdw", bufs=2, space="PSUM") as psdw,
        ):
            w1sb = wpool.tile([128, nblocks * _MH * 256], fp8)
            nc.sync.dma_start(out=w1sb[:], in_=w1_d.ap())
            w2sb = wpool.tile([128, nblocks * _KC * 2 * 256], fp8)
            nc.sync.dma_start(out=w2sb[:], in_=w2_d.ap())
            dwasb = wpool.tile([128, nblocks * _MH * 256], fp8)
            nc.sync.dma_start(out=dwasb[:], in_=dwa_d.ap())
            dwbsb = wpool.tile([128, nblocks * _MH * 128], fp8)
            nc.sync.dma_start(out=dwbsb[:], in_=dwb_d.ap())
            cst = wpool.tile([128, nblocks * _NCC], f32)
            nc.sync.dma_start(out=cst[:], in_=cst_d.ap())

            def w1t(i, mh):
                o = (i * _MH + mh) * 256
                return _mk3(w1sb[:, o:o + 256], 128, 128)

            def w2t(i, mc, pair):
                o = (i * _KC * 2 + mc * 2 + pair) * 256
                return _mk3(w2sb[:, o:o + 256], 128, 128)

            def dwAt(i, mh):
                o = (i * _MH + mh) * 256
                return _mk3(dwasb[:, o:o + 256], 128, 128)

            def dwBt(i, mh):
                o = (i * _MH + mh) * 128
                return dwbsb[:, o:o + 128]

            def cc(i, j):
                return cst[:, i * _NCC + j:i * _NCC + j + 1]

            Rb = {}
            for b in range(bs):
                Rb[b] = []
                for kc in range(_KC):
                    rt = rpool.tile([128, T], f32, tag="R",
                                    name=f"R_b{b}_{kc}")
                    nc.sync.dma_start(
                        out=rt[:], in_=x_d.ap()[b, kc * 128:(kc + 1) * 128, :])
                    Rb[b].append(rt)

            state = {}

            def emitA_alloc(b, i):
                s1 = s1pool.tile([128, _KC * T], fp8, tag="S1",
                                 name=f"S1_b{b}_i{i}")
                # full-T S1: R for this unit was finalized by the previous
                # same-b unit's BC, so one wide 2x-mode op per kc suffices
                for kc in range(_KC):
                    nc.vector.tensor_scalar(
                        s1[:, kc * T:(kc + 1) * T], Rb[b][kc][:],
                        cc(i, kc), None, op0=ALU.is_ge)
                N2 = []
                for mh in range(_MH):
                    n2 = n2pool.tile([128, T + 2 * _PAD], fp8, tag="N2",
                                     name=f"N2_b{b}_i{i}_{mh}")
                    hv = 0.5 if _isN2dve(i, mh) else 0.0
                    nc.gpsimd.memset(n2[:, 0:_PAD], hv)
                    nc.gpsimd.memset(n2[:, _PAD + T:2 * _PAD + T], hv)
                    N2.append(n2)
                state[(b, i)] = (s1, N2)

            def emitA_g(b, i, g):
                s1, N2 = state[(b, i)]
                c0 = g * _QC
                for mh in range(_MH):
                    ps = psmm.tile([128, _QC], f32, tag="mm",
                                   name=f"psA_{b}_{i}_{mh}_{g}")
                    for nn in range(2):
                        cn = c0 + nn * 512
                        rhs = _mk3(s1[:, cn:cn + 512], T, 512)
                        mm = nc.tensor.matmul(
                            ps[:, nn * 512:(nn + 1) * 512],
                            w1t(i, mh), rhs, start=True, stop=True,
                            perf_mode=DRM)
                        if nn == 1:
                            mm.ins.ldweights = False
                    dst = N2[mh][:, _PAD + c0:_PAD + c0 + _QC]
                    if _isN2dve(i, mh):
                        nc.vector.tensor_scalar(
                            dst, ps[:], cc(i, 2 + mh), None, op0=ALU.is_lt)
                    else:
                        nc.scalar.activation(
                            dst, ps[:], ACTF.Sign,
                            bias=cc(i, 2 + mh), scale=-2.0)

            def emitBC_q(b, i, q):
                d = 2 ** i
                R = Rb[b]
                _, N2 = state[(b, i)]
                c0 = q * _QC
                S3 = [s3pool.tile([128, 2 * _QC], fp8, tag="S3",
                                  name=f"S3_b{b}_i{i}_q{q}_p{p}")
                      for p in range(2)]
                for mh in range(_MH):
                    n2 = N2[mh]
                    pd = psdw.tile([128, _QC], f32, tag="dw",
                                   name=f"psD_{b}_{i}_{mh}_{q}")
                    if _isV(i, mh):
                        # grouped A,A,B,B so LDWEIGHTS is reused per pair
                        for nn in range(2):
                            w0 = _PAD + c0 + nn * 512
                            rhs01 = _mk3(n2[:, w0 - d:w0 - d + 512], d, 512)
                            mm = nc.tensor.matmul(
                                pd[:, nn * 512:(nn + 1) * 512],
                                dwAt(i, mh), rhs01,
                                start=True, stop=False, perf_mode=DRM)
                            if nn == 1:
                                mm.ins.ldweights = False
                        for nn in range(2):
                            w0 = _PAD + c0 + nn * 512
                            mm = nc.tensor.matmul(
                                pd[:, nn * 512:(nn + 1) * 512],
                                dwBt(i, mh), n2[:, w0 + d:w0 + d + 512],
                                start=False, stop=True)
                            if nn == 1:
                                mm.ins.ldweights = False
                    else:
                        for nn in range(2):
                            w0 = _PAD + c0 + nn * 512
                            rhs02 = _mk3(n2[:, w0 - d:w0 - d + 512],
                                         2 * d, 512)
                            mm = nc.tensor.matmul(
                                pd[:, nn * 512:(nn + 1) * 512],
                                dwAt(i, mh), rhs02,
                                start=True, stop=True, perf_mode=DRM)
                            if nn == 1:
                                mm.ins.ldweights = False
                    s3out = S3[mh // 2][:, (mh % 2) * _QC:(mh % 2 + 1) * _QC]
                    if _isV(i, mh):
                        sc = 2.0 if _isN2dve(i, mh) else 1.0
                        nc.scalar.activation(
                            s3out, pd[:], ACTF.Sign,
                            bias=cc(i, 6 + mh), scale=sc)
                    else:
                        nc.vector.scalar_tensor_tensor(
                            s3out, pd[:], cc(i, 6 + mh),
                            n2[:, _PAD + c0:_PAD + c0 + _QC],
                            op0=ALU.add, op1=ALU.is_ge)
                for mc in range(_KC):
                    ps2 = psmm.tile([128, _QC], f32, tag="mm",
                                    name=f"psC_{b}_{i}_{q}_{mc}")
                    # pair-outer: reuse each LDWEIGHTS across both nn halves
                    for pair in range(2):
                        for nn in range(2):
                            rhs = _mk3(S3[pair][:, nn * 512:nn * 512 + 512],
                                       _QC, 512)
                            mm = nc.tensor.matmul(
                                ps2[:, nn * 512:(nn + 1) * 512],
                                w2t(i, mc, pair), rhs,
                                start=(pair == 0), stop=(pair == 1),
                                perf_mode=DRM)
                            if nn == 1:
                                mm.ins.ldweights = False
                    nc.vector.scalar_tensor_tensor(
                        R[mc][:, c0:c0 + _QC], ps2[:],
                        cc(i, 10 + mc), R[mc][:, c0:c0 + _QC],
                        op0=ALU.add, op1=ALU.add)

            # software-pipelined emission: stage A of unit k+1 interleaves
            # chunk-wise with stage B/C of unit k (independent units).
            # Prologue pipelines within unit 0: BC_q(0, q) only needs N2
            # chunks q-1..q+1, so it starts after A_g(0, q+1).
            seq = [(b, i) for i in range(nblocks) for b in range(bs)]
            emitA_alloc(*seq[0])
            emitA_g(*seq[0], 0)
            emitA_g(*seq[0], 1)
            emitA_alloc(*seq[1])
            emitA_g(*seq[1], 0)
            emitBC_q(*seq[0], 0)
            emitA_g(*seq[0], 2)
            emitA_g(*seq[1], 1)
            emitBC_q(*seq[0], 1)
            emitA_g(*seq[0], 3)
            emitA_g(*seq[1], 2)
            emitBC_q(*seq[0], 2)
            emitA_g(*seq[1], 3)
            emitBC_q(*seq[0], 3)
            state.pop(seq[0])
            for k in range(1, len(seq)):
                if k + 1 < len(seq):
                    emitA_alloc(*seq[k + 1])
                for g in range(nq):
                    emitBC_q(*seq[k], g)
                    if k + 1 < len(seq):
                        emitA_g(*seq[k + 1], g)
                state.pop(seq[k])

            for b in range(bs):
                for kc in range(_KC):
                    nc.sync.dma_start(
                        out=out_d.ap()[b, kc * 128:(kc + 1) * 128, :],
                        in_=Rb[b][kc][:])
    nc.finalize()
    return nc


def _prep(inputs, nblocks=_BLOCKS):
    """Host-side weight/threshold preprocessing (tiny tensors only)."""
    e4 = ml_dtypes.float8_e4m3

    def thr(g, bb, m, v):
        return (m - bb * np.sqrt(v + _EPS) / g).astype(np.float32)

    w1dr = np.zeros((128, nblocks * _MH * 256), np.float32)
    w2dr = np.zeros((128, nblocks * _KC * 2 * 256), np.float32)
    dwA = np.zeros((128, nblocks * _MH * 256), np.float32)
    dwB = np.zeros((128, nblocks * _MH * 128), np.float32)
    cst = np.zeros((128, nblocks * _NCC), np.float32)
    ar = np.arange(128)
    for i in range(nblocks):
        t1 = thr(inputs['bn1_gamma'][i], inputs['bn1_beta'][i],
                 inputs['bn1_mean'][i], inputs['bn1_var'][i])          # [Cb]
        t2 = thr(inputs['bn2_gamma'][i], inputs['bn2_beta'][i],
                 inputs['bn2_mean'][i], inputs['bn2_var'][i])          # [H]
        t3 = thr(inputs['bn3_gamma'][i], inputs['bn3_beta'][i],
                 inputs['bn3_mean'][i], inputs['bn3_var'][i])          # [H]
        W1s = np.sign(inputs['w1'][i]).astype(np.float32)              # [H, Cb]
        W2s = np.sign(inputs['w2'][i]).astype(np.float32)              # [Cb, H]
        dws = np.sign(inputs['dw_w'][i]).astype(np.float32)            # [H, 3]
        ctr = dws[:, 1]
        a0 = dws[:, 0] * ctr
        a2 = dws[:, 2] * ctr
        rs1 = W1s.sum(axis=1)                                          # [H]
        tau3 = ctr * t3
        # U tiles ({0,1}-encoded S3) get amp 2; V tiles (+-1 S3) amp 1
        mh_of = np.arange(_H) // 128
        isV = np.array([_isV(i, mh) for mh in mh_of])
        amp = np.where(isV, 1.0, 2.0)
        W2x = W2s * (ctr * amp)[None, :]                               # [Cb, H]
        for mh in range(_MH):
            o = (i * _MH + mh) * 256
            for j in range(2):
                w1dr[:, o + j * 128:o + (j + 1) * 128] = \
                    W1s[mh * 128:(mh + 1) * 128, j * 128:(j + 1) * 128].T
        for mc in range(_KC):
            for pair in range(2):
                o = (i * _KC * 2 + mc * 2 + pair) * 256
                for j in range(2):
                    kh = pair * 2 + j
                    w2dr[:, o + j * 128:o + (j + 1) * 128] = \
                        W2x[mc * 128:(mc + 1) * 128,
                            kh * 128:(kh + 1) * 128].T
        for mh in range(_MH):
            sl = slice(mh * 128, (mh + 1) * 128)
            o = (i * _MH + mh) * 256
            o2 = (i * _MH + mh) * 128
            if _isV(i, mh):
                # 3-tap path: DR rows (t-d, t) coeffs (-a0, -1); plain (+d): -a2
                dwA[ar, o + ar] = -a0[sl]
                dwA[ar, o + 128 + ar] = -1.0
                dwB[ar, o2 + ar] = -a2[sl]
            else:
                # 2-tap path: DR rows (t-d, t+d) coeffs (-a0, -a2)
                dwA[ar, o + ar] = -a0[sl]
                dwA[ar, o + 128 + ar] = -a2[sl]
        base = i * _NCC
        for kc in range(_KC):
            cst[:, base + kc] = t1[kc * 128:(kc + 1) * 128]
        for mh in range(_MH):
            sl = slice(mh * 128, (mh + 1) * 128)
            nb = rs1[sl] + t2[sl]
            # N2 on ACT: Sign(-2*p1' + nb); on DVE: is_lt(p1', nb/2)
            cst[:, base + 2 + mh] = (nb / 2 if _isN2dve(i, mh) else nb)
            if _isV(i, mh):
                if _isN2dve(i, mh):
                    cst[:, base + 6 + mh] = (a0[sl] + 1 + a2[sl]) - tau3[sl]
                else:
                    cst[:, base + 6 + mh] = -tau3[sl]
            else:
                if _isN2dve(i, mh):
                    cst[:, base + 6 + mh] = \
                        (a0[sl] + a2[sl] + 1 - tau3[sl]) / 2
                else:
                    cst[:, base + 6 + mh] = -tau3[sl]
        # R correction: U channels contribute +W2s*ctr per column extra
        rn = -(W2s * (ctr * (~isV))[None, :]).sum(axis=1)              # [Cb]
        for mc in range(_KC):
            cst[:, base + 10 + mc] = rn[mc * 128:(mc + 1) * 128]
    return (w1dr.astype(e4), w2dr.astype(e4), dwA.astype(e4),
            dwB.astype(e4), cst)


def kernel(**inputs):
    inputs = {k: np.asarray(v) for k, v in inputs.items()}
    x = inputs['x'].astype(np.float32)
    w1dr, w2dr, dwA, dwB, cst = _prep(inputs)

    if 'nc' not in _nc_cache:
        _nc_cache['nc'] = _build_nc()
    nc = _nc_cache['nc']

    in_maps = []
    for c in range(_NCORES):
        in_maps.append({
            'x': np.ascontiguousarray(x[c * _BS:(c + 1) * _BS]),
            'w1dr': w1dr, 'w2dr': w2dr, 'dwA': dwA, 'dwB': dwB,
            'cst': cst,
        })

    from concourse.bass_utils import run_bass_kernel_spmd
    import os
    trace = bool(int(os.environ.get('KERNEL_TRACE', '0')))
    res = run_bass_kernel_spmd(nc, in_maps, core_ids=list(range(_NCORES)),
                               trace=trace)
    _nc_cache['last_result'] = res
    out = np.concatenate([r['out'] for r in res.results], axis=0)
    return out.astype(np.float32)


# revision 21
# speedup vs baseline: 3.8768x; 1.0869x over previous
"""Bass/Trainium2 kernel for nn_BitwiseTasNetRepeat.

Strategy (v4: STT-fused center tap + STT-fused residual correction)
-------------------------------------------------------------------
Each sign(BN(.)) collapses to a per-channel threshold compare. Per block:

    S1 = (R >= t1)                          {0,1} fp8  (DVE is_ge, 2x mode)
    p1 = W1s @ S1                           (TensorE fp8 DR, K=256)
    N2 = Sign(-2*p1 + (rs1+t2)) = -sign(bn2)  (ACT, fp8 +-1)
    U-path (S3 on DVE):
      q0 = -a0*N2(-d) - a2*N2(+d)           (ONE diag DR matmul, taps +-d)
      S3 = (q0 - tau3 >= N2(0))             {0,1}  (DVE scalar_tensor_tensor)
    V-path (S3 on ACT):
      qv = -a0*N2(-d) - N2(0) - a2*N2(+d)   (diag DR + diag plain matmul)
      S3 = Sign(qv - tau3)                  +-1    (ACT)
    ps2 = W2x @ S3                          (TensorE fp8 DR, K=512)
    R   = (ps2 + rneg) + R                  (DVE scalar_tensor_tensor;
                                             rneg corrects U's {0,1} encoding)

All values exact in fp8e4m3 / fp32-PSUM; result is bit-exact.
Sharding: data-parallel over batch, 2 batches per core, 8 cores.
Loops are ordered (stage-major, b-outer, mh-outer) so TensorE reuses
LDWEIGHTS across 8-16 matmuls and every engine FIFO stays dependency-
feasible in emission order.
"""

import numpy as np
import ml_dtypes

_B, _CB, _H, _T = 16, 256, 512, 4096
_BLOCKS = 8
_EPS = 1e-5
_NCORES = 8
_BS = _B // _NCORES      # batches per core
_KC = _CB // 128         # 2  k-tiles of Cb
_MH = _H // 128          # 4  m-tiles of H
_PAD = 128               # halo for dilated depthwise conv (max d = 128)
_NCC = 12                # f32 const columns per block
_QC = 1024               # chunk width

# --- engine assignment knobs (tunable) ---------------------------------
def _isV(i, mh):
    """True -> S3 of tile (i, mh) runs on ACT via the 3-tap matmul path
    ("V"); False -> 2-tap matmul + DVE scalar_tensor_tensor ("U")."""
    return mh < 2


def _isN2dve(i, mh):
    """True -> N2 of tile (i, mh) produced on DVE as {0,1} (is_lt);
    False -> on ACT as +-1 (Sign)."""
    return False

_nc_cache = {}


def _mk3(ap2d, j_step, cols):
    """3D AP [128, 2 (stride j_step), cols] over a 2D row view."""
    import bass_rust
    v = ap2d.copy()
    l = v.ap
    v.ap = bass_rust.VecI64Pair([list(l[0]), [j_step, 2], [1, cols]])
    return v


def _build_nc(bs=_BS, nblocks=_BLOCKS, T=_T):
    import concourse.mybir as mybir
    from concourse import bacc
    from concourse.tile import TileContext

    f32 = mybir.dt.float32
    fp8 = mybir.dt.float8e4
    ALU = mybir.AluOpType
    ACTF = mybir.ActivationFunctionType
    DRM = mybir.MatmulPerfMode.DoubleRow
    nq = T // _QC

    nc = bacc.Bacc("TRN2", target_bir_lowering=False, debug=False,
                   enable_asserts=False)

    x_d = nc.dram_tensor("x", [bs, _CB, T], f32, kind="ExternalInput")
    w1_d = nc.dram_tensor("w1dr", [128, nblocks * _MH * 256], fp8,
                          kind="ExternalInput")
    w2_d = nc.dram_tensor("w2dr", [128, nblocks * _KC * 2 * 256], fp8,
                          kind="ExternalInput")
    dwa_d = nc.dram_tensor("dwA", [128, nblocks * _MH * 256], fp8,
                           kind="ExternalInput")
    dwb_d = nc.dram_tensor("dwB", [128, nblocks * _MH * 128], fp8,
                           kind="ExternalInput")
    cst_d = nc.dram_tensor("cst", [128, nblocks * _NCC], f32,
                           kind="ExternalInput")
    out_d = nc.dram_tensor("out", [bs, _CB, T], f32, kind="ExternalOutput")

    with TileContext(nc) as tc:
        with (
            tc.tile_pool(name="wpool", bufs=1) as wpool,
            tc.tile_pool(name="rpool", bufs=4) as rpool,
            tc.tile_pool(name="s1pool", bufs=3) as s1pool,
            tc.tile_pool(name="n2pool", bufs=8) as n2pool,
            tc.tile_pool(name="s3pool", bufs=8) as s3pool,
            tc.tile_pool(name="psmm", bufs=2, space="PSUM") as psmm,
            tc.tile_pool(name="psdw", bufs=2, space="PSUM") as psdw,
        ):
            w1sb = wpool.tile([128, nblocks * _MH * 256], fp8)
            nc.sync.dma_start(out=w1sb[:], in_=w1_d.ap())
            w2sb = wpool.tile([128, nblocks * _KC * 2 * 256], fp8)
            nc.sync.dma_start(out=w2sb[:], in_=w2_d.ap())
            dwasb = wpool.tile([128, nblocks * _MH * 256], fp8)
            nc.sync.dma_start(out=dwasb[:], in_=dwa_d.ap())
            dwbsb = wpool.tile([128, nblocks * _MH * 128], fp8)
            nc.sync.dma_start(out=dwbsb[:], in_=dwb_d.ap())
            cst = wpool.tile([128, nblocks * _NCC], f32)
            nc.sync.dma_start(out=cst[:], in_=cst_d.ap())

            def w1t(i, mh):
                o = (i * _MH + mh) * 256
                return _mk3(w1sb[:, o:o + 256], 128, 128)

            def w2t(i, mc, pair):
                o = (i * _KC * 2 + mc * 2 + pair) * 256
                return _mk3(w2sb[:, o:o + 256], 128, 128)

            def dwAt(i, mh):
                o = (i * _MH + mh) * 256
                return _mk3(dwasb[:, o:o + 256], 128, 128)

            def dwBt(i, mh):
                o = (i * _MH + mh) * 128
                return dwbsb[:, o:o + 128]

            def cc(i, j):
                return cst[:, i * _NCC + j:i * _NCC + j + 1]

            Rb = {}
            for b in range(bs):
                Rb[b] = []
                for kc in range(_KC):
                    rt = rpool.tile([128, T], f32, tag="R",
                                    name=f"R_b{b}_{kc}")
                    nc.sync.dma_start(
                        out=rt[:], in_=x_d.ap()[b, kc * 128:(kc + 1) * 128, :])
                    Rb[b].append(rt)

            # persistent N2 tiles: halos memset once (always zero)
            N2p = {}
            for b in range(bs):
                for mh in range(_MH):
                    n2 = n2pool.tile([128, T + 2 * _PAD], fp8, tag="N2",
                                     name=f"N2_b{b}_{mh}")
                    nc.gpsimd.memset(n2[:, 0:_PAD], 0.0)
                    nc.gpsimd.memset(n2[:, _PAD + T:2 * _PAD + T], 0.0)
                    N2p[(b, mh)] = n2

            state = {}

            def emitA_alloc(b, i):
                s1 = s1pool.tile([128, _KC * T], fp8, tag="S1",
                                 name=f"S1_b{b}_i{i}")
                N2 = [N2p[(b, mh)] for mh in range(_MH)]
                state[(b, i)] = (s1, N2)

            def emitA_g(b, i, g):
                s1, N2 = state[(b, i)]
                c0 = g * _QC
                for kc in range(_KC):
                    nc.vector.tensor_scalar(
                        s1[:, kc * T + c0:kc * T + c0 + _QC],
                        Rb[b][kc][:, c0:c0 + _QC],
                        cc(i, kc), None, op0=ALU.is_ge)
                for mh in range(_MH):
                    ps = psmm.tile([128, _QC], f32, tag="mm",
                                   name=f"psA_{b}_{i}_{mh}_{g}")
                    for nn in range(2):
                        cn = c0 + nn * 512
                        rhs = _mk3(s1[:, cn:cn + 512], T, 512)
                        nc.tensor.matmul(
                            ps[:, nn * 512:(nn + 1) * 512],
                            w1t(i, mh), rhs, start=True, stop=True,
                            perf_mode=DRM)
                    dst = N2[mh][:, _PAD + c0:_PAD + c0 + _QC]
                    if _isN2dve(i, mh):
                        nc.vector.tensor_scalar(
                            dst, ps[:], cc(i, 2 + mh), None, op0=ALU.is_lt)
                    else:
                        nc.scalar.activation(
                            dst, ps[:], ACTF.Sign,
                            bias=cc(i, 2 + mh), scale=-2.0)

            def emitBC_q(b, i, q):
                d = 2 ** i
                R = Rb[b]
                _, N2 = state[(b, i)]
                c0 = q * _QC
                S3 = [s3pool.tile([128, 2 * _QC], fp8, tag="S3",
                                  name=f"S3_b{b}_i{i}_q{q}_p{p}")
                      for p in range(2)]
                for mh in range(_MH):
                    n2 = N2[mh]
                    pd = psdw.tile([128, _QC], f32, tag="dw",
                                   name=f"psD_{b}_{i}_{mh}_{q}")
                    if _isV(i, mh):
                        # grouped A,A,B,B so LDWEIGHTS is reused per pair
                        for nn in range(2):
                            w0 = _PAD + c0 + nn * 512
                            rhs01 = _mk3(n2[:, w0 - d:w0 - d + 512], d, 512)
                            nc.tensor.matmul(
                                pd[:, nn * 512:(nn + 1) * 512],
                                dwAt(i, mh), rhs01,
                                start=True, stop=False, perf_mode=DRM)
                        for nn in range(2):
                            w0 = _PAD + c0 + nn * 512
                            nc.tensor.matmul(
                                pd[:, nn * 512:(nn + 1) * 512],
                                dwBt(i, mh), n2[:, w0 + d:w0 + d + 512],
                                start=False, stop=True)
                    else:
                        for nn in range(2):
                            w0 = _PAD + c0 + nn * 512
                            rhs02 = _mk3(n2[:, w0 - d:w0 - d + 512],
                                         2 * d, 512)
                            nc.tensor.matmul(
                                pd[:, nn * 512:(nn + 1) * 512],
                                dwAt(i, mh), rhs02,
                                start=True, stop=True, perf_mode=DRM)
                    s3out = S3[mh // 2][:, (mh % 2) * _QC:(mh % 2 + 1) * _QC]
                    if _isV(i, mh):
                        sc = 2.0 if _isN2dve(i, mh) else 1.0
                        nc.scalar.activation(
                            s3out, pd[:], ACTF.Sign,
                            bias=cc(i, 6 + mh), scale=sc)
                    else:
                        nc.vector.scalar_tensor_tensor(
                            s3out, pd[:], cc(i, 6 + mh),
                            n2[:, _PAD + c0:_PAD + c0 + _QC],
                            op0=ALU.add, op1=ALU.is_ge)
                for mc in range(_KC):
                    ps2 = psmm.tile([128, _QC], f32, tag="mm",
                                    name=f"psC_{b}_{i}_{q}_{mc}")
                    # pair-outer: reuse each LDWEIGHTS across both nn halves
                    for pair in range(2):
                        for nn in range(2):
                            rhs = _mk3(S3[pair][:, nn * 512:nn * 512 + 512],
                                       _QC, 512)
                            nc.tensor.matmul(
                                ps2[:, nn * 512:(nn + 1) * 512],
                                w2t(i, mc, pair), rhs,
                                start=(pair == 0), stop=(pair == 1),
                                perf_mode=DRM)
                    nc.vector.scalar_tensor_tensor(
                        R[mc][:, c0:c0 + _QC], ps2[:],
                        cc(i, 10 + mc), R[mc][:, c0:c0 + _QC],
                        op0=ALU.add, op1=ALU.add)

            # software-pipelined emission: stage A of unit k+1 interleaves
            # chunk-wise with stage B/C of unit k (independent units).
            # Prologue pipelines within unit 0: BC_q(0, q) only needs N2
            # chunks q-1..q+1, so it starts after A_g(0, q+1).
            seq = [(b, i) for i in range(nblocks) for b in range(bs)]
            emitA_alloc(*seq[0])
            emitA_g(*seq[0], 0)
            emitA_g(*seq[0], 1)
            emitA_alloc(*seq[1])
            emitA_g(*seq[1], 0)
            emitBC_q(*seq[0], 0)
            emitA_g(*seq[0], 2)
            emitA_g(*seq[1], 1)
            emitBC_q(*seq[0], 1)
            emitA_g(*seq[0], 3)
            emitA_g(*seq[1], 2)
            emitBC_q(*seq[0], 2)
            emitA_g(*seq[1], 3)
            emitBC_q(*seq[0], 3)
            state.pop(seq[0])
            for k in range(1, len(seq)):
                if k + 1 < len(seq):
                    emitA_alloc(*seq[k + 1])
                for g in range(nq):
                    emitBC_q(*seq[k], g)
                    if k + 1 < len(seq):
                        emitA_g(*seq[k + 1], g)
                state.pop(seq[k])

            for b in range(bs):
                for kc in range(_KC):
                    nc.sync.dma_start(
                        out=out_d.ap()[b, kc * 128:(kc + 1) * 128, :],
                        in_=Rb[b][kc][:])
    nc.finalize()
    return nc


def _prep(inputs, nblocks=_BLOCKS):
    """Host-side weight/threshold preprocessing (tiny tensors only)."""
    e4 = ml_dtypes.float8_e4m3

    def thr(g, bb, m, v):
        return (m - bb * np.sqrt(v + _EPS) / g).astype(np.float32)

    w1dr = np.zeros((128, nblocks * _MH * 256), np.float32)
    w2dr = np.zeros((128, nblocks * _KC * 2 * 256), np.float32)
    dwA = np.zeros((128, nblocks * _MH * 256), np.float32)
    dwB = np.zeros((128, nblocks * _MH * 128), np.float32)
    cst = np.zeros((128, nblocks * _NCC), np.float32)
    ar = np.arange(128)
    for i in range(nblocks):
        t1 = thr(inputs['bn1_gamma'][i], inputs['bn1_beta'][i],
                 inputs['bn1_mean'][i], inputs['bn1_var'][i])          # [Cb]
        t2 = thr(inputs['bn2_gamma'][i], inputs['bn2_beta'][i],
                 inputs['bn2_mean'][i], inputs['bn2_var'][i])          # [H]
        t3 = thr(inputs['bn3_gamma'][i], inputs['bn3_beta'][i],
                 inputs['bn3_mean'][i], inputs['bn3_var'][i])          # [H]
        W1s = np.sign(inputs['w1'][i]).astype(np.float32)              # [H, Cb]
        W2s = np.sign(inputs['w2'][i]).astype(np.float32)              # [Cb, H]
        dws = np.sign(inputs['dw_w'][i]).astype(np.float32)            # [H, 3]
        ctr = dws[:, 1]
        a0 = dws[:, 0] * ctr
        a2 = dws[:, 2] * ctr
        rs1 = W1s.sum(axis=1)                                          # [H]
        tau3 = ctr * t3
        # U tiles ({0,1}-encoded S3) get amp 2; V tiles (+-1 S3) amp 1
        mh_of = np.arange(_H) // 128
        isV = np.array([_isV(i, mh) for mh in mh_of])
        amp = np.where(isV, 1.0, 2.0)
        W2x = W2s * (ctr * amp)[None, :]                               # [Cb, H]
        for mh in range(_MH):
            o = (i * _MH + mh) * 256
            for j in range(2):
                w1dr[:, o + j * 128:o + (j + 1) * 128] = \
                    W1s[mh * 128:(mh + 1) * 128, j * 128:(j + 1) * 128].T
        for mc in range(_KC):
            for pair in range(2):
                o = (i * _KC * 2 + mc * 2 + pair) * 256
                for j in range(2):
                    kh = pair * 2 + j
                    w2dr[:, o + j * 128:o + (j + 1) * 128] = \
                        W2x[mc * 128:(mc + 1) * 128,
                            kh * 128:(kh + 1) * 128].T
        for mh in range(_MH):
            sl = slice(mh * 128, (mh + 1) * 128)
            o = (i * _MH + mh) * 256
            o2 = (i * _MH + mh) * 128
            if _isV(i, mh):
                # 3-tap path: DR rows (t-d, t) coeffs (-a0, -1); plain (+d): -a2
                dwA[ar, o + ar] = -a0[sl]
                dwA[ar, o + 128 + ar] = -1.0
                dwB[ar, o2 + ar] = -a2[sl]
            else:
                # 2-tap path: DR rows (t-d, t+d) coeffs (-a0, -a2)
                dwA[ar, o + ar] = -a0[sl]
                dwA[ar, o + 128 + ar] = -a2[sl]
        base = i * _NCC
        for kc in range(_KC):
            cst[:, base + kc] = t1[kc * 128:(kc + 1) * 128]
        for mh in range(_MH):
            sl = slice(mh * 128, (mh + 1) * 128)
            nb = rs1[sl] + t2[sl]
            # N2 on ACT: Sign(-2*p1' + nb); on DVE: is_lt(p1', nb/2)
            cst[:, base + 2 + mh] = (nb / 2 if _isN2dve(i, mh) else nb)
            if _isV(i, mh):
                if _isN2dve(i, mh):
                    cst[:, base + 6 + mh] = (a0[sl] + 1 + a2[sl]) - tau3[sl]
                else:
                    cst[:, base + 6 + mh] = -tau3[sl]
            else:
                if _isN2dve(i, mh):
                    cst[:, base + 6 + mh] = \
                        (a0[sl] + a2[sl] + 1 - tau3[sl]) / 2
                else:
                    cst[:, base + 6 + mh] = -tau3[sl]
        # R correction: U channels contribute +W2s*ctr per column extra
        rn = -(W2s * (ctr * (~isV))[None, :]).sum(axis=1)              # [Cb]
        for mc in range(_KC):
            cst[:, base + 10 + mc] = rn[mc * 128:(mc + 1) * 128]
    return (w1dr.astype(e4), w2dr.astype(e4), dwA.astype(e4),
            dwB.astype(e4), cst)


def kernel(**inputs):
    inputs = {k: np.asarray(v) for k, v in inputs.items()}
    x = inputs['x'].astype(np.float32)
    w1dr, w2dr, dwA, dwB, cst = _prep(inputs)

    if 'nc' not in _nc_cache:
        _nc_cache['nc'] = _build_nc()
    nc = _nc_cache['nc']

    in_maps = []
    for c in range(_NCORES):
        in_maps.append({
            'x': np.ascontiguousarray(x[c * _BS:(c + 1) * _BS]),
            'w1dr': w1dr, 'w2dr': w2dr, 'dwA': dwA, 'dwB': dwB,
            'cst': cst,
        })

    from concourse.bass_utils import run_bass_kernel_spmd
    import os
    trace = bool(int(os.environ.get('KERNEL_TRACE', '0')))
    res = run_bass_kernel_spmd(nc, in_maps, core_ids=list(range(_NCORES)),
                               trace=trace)
    _nc_cache['last_result'] = res
    out = np.concatenate([r['out'] for r in res.results], axis=0)
    return out.astype(np.float32)
